# revision 2
# baseline (speedup 1.0000x reference)
"""Trainium2 Bass kernel for nn_AttentionBlock (B=2, T=4096, C=512, H=8 causal
attention, fused qkv projection), SPMD across 8 NeuronCores.

Sharding: core c = (batch b = c//4, head-pair g = c%4). Data parallel on B,
tensor parallel splitting the 8 heads (2 per core) and the qkv projection
columns. Each core computes its own T x T score slabs tile-by-tile (flash
style: scores stay in PSUM/SBUF, never hit HBM).

Per-core dataflow (all matmul operands bf16, accumulation fp32):
  qT/kT/vT = w_s^T @ x^T   (PE, K=512 in 4 chunks; bias on DVE)
  V natural via PE transpose, augmented with a ones column so the softmax
  denominator rides the PV matmul for free.
  Per 512-wide query block I:  S^T tiles [128 keys, 512 queries] on PE,
  additive causal mask on DVE (diagonal tiles), exp on ScalarE (scale=1/8
  fused, bf16 out), P^T@V accumulated into [65, 512] PSUM (row 64 = denom).
  Normalize: PE transpose back, reciprocal * scale on DVE, DMA out fp32.
Projection for block I+1 is emitted mid-block-I so PE/ScalarE stay busy.
"""
from contextlib import ExitStack

import numpy as np
import ml_dtypes

import concourse.bass as bass
import concourse.mybir as mybir
from concourse import bacc
from concourse.tile import TileContext
from concourse.masks import make_identity

F32 = mybir.dt.float32
BF16 = mybir.dt.bfloat16
Exp = mybir.ActivationFunctionType.Exp

B = 2
T = 4096
C = 512
H = 8
HD = 64
NCK = 4          # contraction chunks of 128
TB = 512         # query tile (psum free dim)
JB = 128         # key block (partitions)
GJ = 2           # key blocks per exp batch
SCALE = 0.125    # 1/sqrt(HD)


def _build():
    nI = T // TB
    nT128 = T // JB
    NSUB = TB // JB

    nc = bacc.Bacc("TRN2", target_bir_lowering=False, debug=False)
    xT_d = nc.dram_tensor("xT", [C, T], BF16, kind="ExternalInput")
    w_d = nc.dram_tensor("w", [NCK, 128, 384], BF16, kind="ExternalInput")
    b_d = nc.dram_tensor("bias", [3, 128, 1], F32, kind="ExternalInput")
    out_d = nc.dram_tensor("out", [T, 128], F32, kind="ExternalOutput")

    with TileContext(nc) as tc, ExitStack() as stk:
        pp = stk.enter_context(tc.tile_pool(name="persist", bufs=1))
        st_ps = stk.enter_context(tc.tile_pool(name="st_ps", bufs=2, space="PSUM"))
        ot_ps = stk.enter_context(tc.tile_pool(name="ot_ps", bufs=1, space="PSUM"))
        otr_ps = stk.enter_context(tc.tile_pool(name="otr_ps", bufs=1, space="PSUM"))
        vtr_ps = stk.enter_context(tc.tile_pool(name="vtr_ps", bufs=1, space="PSUM"))
        pt_pool = stk.enter_context(tc.tile_pool(name="pt_pool", bufs=4))
        ob_pool = stk.enter_context(tc.tile_pool(name="ob_pool", bufs=2))
        fin_pool = stk.enter_context(tc.tile_pool(name="fin_pool", bufs=3))

        xT_sb = pp.tile([128, NCK, T], BF16)
        w_sb = pp.tile([128, NCK, 384], BF16)
        bias_sb = pp.tile([128, 3], F32)
        qT_sb = pp.tile([128, T], BF16)
        kT_sb = pp.tile([128, T], BF16)
        vT_sb = pp.tile([128, T], BF16)
        v_sb = [pp.tile([128, nT128, 65], BF16, tag=f"v{h}", name=f"v{h}")
                for h in (0, 1)]
        ident_b = pp.tile([128, 128], BF16)
        ident_f = pp.tile([128, 128], F32)
        ones_b = pp.tile([128, 1], BF16)
        masks = pp.tile([128, NSUB, TB], F32)

        nc.sync.dma_start(w_sb[:], w_d[:].rearrange("a b c -> b a c"))
        for s in range(3):
            nc.gpsimd.dma_start(bias_sb[:, s:s + 1], b_d[s])
        split = min(2 * TB, T)
        for ck in range(NCK):
            nc.sync.dma_start(
                xT_sb[:, ck, 0:split], xT_d[ck * 128:(ck + 1) * 128, 0:split]
            )
        if split < T:
            for ck in range(NCK):
                nc.sync.dma_start(
                    xT_sb[:, ck, split:T], xT_d[ck * 128:(ck + 1) * 128, split:T]
                )

        make_identity(nc, ident_f[:])
        nc.vector.tensor_copy(ident_b[:], ident_f[:])
        nc.vector.memset(ones_b[:], 1.0)
        nc.gpsimd.memset(masks[:], 0.0)
        for p in range(NSUB):
            nc.gpsimd.affine_select(
                out=masks[:, p, :], in_=masks[:, p, :],
                compare_op=mybir.AluOpType.is_ge,
                fill=-1e9,
                base=-JB * p,
                pattern=[[1, TB]],
                channel_multiplier=-1,
            )
        for h in (0, 1):
            nc.vector.tensor_copy(
                v_sb[h][:, :, 64], ones_b[:].broadcast_to([128, nT128])
            )

        dests = [qT_sb, kT_sb, vT_sb]

        def proj(tb):
            for s in (0, 1, 2):
                ps = st_ps.tile([128, GJ, TB], F32, tag="st", name="st")
                for ck in range(NCK):
                    nc.tensor.matmul(
                        ps[:, 0, :],
                        w_sb[:, ck, s * 128:(s + 1) * 128],
                        xT_sb[:, ck, tb * TB:(tb + 1) * TB],
                        start=(ck == 0),
                        stop=(ck == NCK - 1),
                    )
                nc.vector.tensor_scalar_add(
                    dests[s][:, tb * TB:(tb + 1) * TB], ps[:, 0, :],
                    bias_sb[:, s:s + 1],
                )
            for sub in range(NSUB):
                t128 = tb * NSUB + sub
                tp = vtr_ps.tile([128, 128], BF16)
                nc.tensor.transpose(
                    tp[:], vT_sb[:, t128 * 128:(t128 + 1) * 128], ident_b[:]
                )
                for h in (0, 1):
                    nc.vector.tensor_copy(
                        v_sb[h][:, t128, 0:64], tp[:, h * 64:h * 64 + 64]
                    )

        proj(0)
        for I in range(nI):
            jmax = NSUB * (I + 1)
            ots = [ot_ps.tile([65, TB], F32, tag=f"ot{h}", name=f"ot{h}")
                   for h in (0, 1)]
            prev = None

            def flush_pv(prev):
                pJg, ph, pptb, pdiag = prev
                for u in range(GJ):
                    J = pJg * GJ + u
                    p = pdiag[u]
                    lo = JB * p if p >= 1 else 0
                    nc.tensor.matmul(
                        ots[ph][:, lo:],
                        v_sb[ph][:, J, :],
                        pptb[:, u, lo:],
                        start=(J == 0), stop=(J == jmax - 1),
                    )

            ngroups = jmax // GJ
            proj_at = max(1, ngroups // 2)
            for Jg in range(ngroups):
                if Jg == proj_at and I + 1 < nI:
                    proj(I + 1)
                # S matmuls head-interleaved: adjacent PE instructions hit
                # different row groups (K=64 each) and run concurrently.
                stbs = [st_ps.tile([128, GJ, TB], F32, tag="st", name="st")
                        for _ in (0, 1)]
                diag = []
                for u in range(GJ):
                    J = Jg * GJ + u
                    p = J - NSUB * I
                    diag.append(p)
                    lo = JB * p if p >= 1 else 0
                    for h in (0, 1):
                        nc.tensor.matmul(
                            stbs[h][:, u, lo:],
                            kT_sb[h * 64:(h + 1) * 64, J * JB:(J + 1) * JB],
                            qT_sb[h * 64:(h + 1) * 64,
                                  I * TB + lo:(I + 1) * TB],
                            start=True, stop=True,
                        )
                for h in (0, 1):
                    for u in range(GJ):
                        p = diag[u]
                        if p >= 0:
                            nc.vector.tensor_add(
                                stbs[h][:, u, :], stbs[h][:, u, :],
                                masks[:, p, :]
                            )
                    ptb = pt_pool.tile([128, GJ, TB], BF16)
                    nc.scalar.activation(ptb[:], stbs[h][:], Exp, scale=SCALE)
                    if prev is not None:
                        flush_pv(prev)
                    prev = (Jg, h, ptb, diag)
            flush_pv(prev)
            for h in (0, 1):
                ob = ob_pool.tile([65, TB], BF16, tag="ob", name="ob")
                nc.vector.tensor_copy(ob[:], ots[h][:])
                for cp in range(NSUB):
                    tp = otr_ps.tile([128, 65], BF16, tag="tpb", name="tpb")
                    nc.tensor.transpose(
                        tp[:], ob[:, cp * 128:(cp + 1) * 128],
                        ident_b[0:65, 0:65],
                    )
                    rec = fin_pool.tile([128, 1], F32, tag="rec")
                    nc.vector.reciprocal(rec[:], tp[:, 64:65])
                    ofin = fin_pool.tile([128, 64], F32, tag="ofin")
                    nc.vector.tensor_scalar_mul(ofin[:], tp[:, 0:64], rec[:])
                    nc.sync.dma_start(
                        out_d[I * TB + cp * 128: I * TB + (cp + 1) * 128,
                              h * 64:(h + 1) * 64],
                        ofin[:],
                    )
    nc.compile()
    return nc


def _core_inputs(x_b, w_qkv, b_qkv, g):
    cols = np.concatenate([
        np.arange(128 * g, 128 * (g + 1)),
        512 + np.arange(128 * g, 128 * (g + 1)),
        1024 + np.arange(128 * g, 128 * (g + 1)),
    ])
    w4 = np.ascontiguousarray(
        w_qkv[:, cols].reshape(NCK, 128, 384).astype(ml_dtypes.bfloat16))
    bias = np.ascontiguousarray(
        b_qkv[cols].reshape(3, 128, 1).astype(np.float32))
    xT = np.ascontiguousarray(x_b.T.astype(ml_dtypes.bfloat16))
    return {"xT": xT, "w": w4, "bias": bias}


class _Runner:
    """Build the jitted SPMD callable once; reuse across kernel() calls."""

    def __init__(self, nc, n_cores=8):
        import jax
        from jax.sharding import Mesh, PartitionSpec, NamedSharding
        from jax.experimental.shard_map import shard_map
        from concourse.bass2jax import (
            _bass_exec_p, install_neuronx_cc_hook, partition_id_tensor,
        )
        install_neuronx_cc_hook()
        self.jax = jax
        partition_name = (
            nc.partition_id_tensor.name if nc.partition_id_tensor else None
        )
        in_names, out_names, out_avals, zero_shapes = [], [], [], []
        for alloc in nc.m.functions[0].allocations:
            if not isinstance(alloc, mybir.MemoryLocationSet):
                continue
            name = alloc.memorylocations[0].name
            if alloc.kind == "ExternalInput":
                if name != partition_name:
                    in_names.append(name)
            elif alloc.kind == "ExternalOutput":
                shape = tuple(alloc.tensor_shape)
                dtype = mybir.dt.np(alloc.dtype)
                out_names.append(name)
                out_avals.append(jax.core.ShapedArray(shape, dtype))
                zero_shapes.append((shape, dtype))
        self.in_names = in_names
        self.out_names = out_names
        self.out_avals = out_avals
        self.n_cores = n_cores
        all_in = list(in_names) + list(out_names)
        if partition_name is not None:
            all_in.append(partition_name)

        def _body(*args):
            operands = list(args)
            if partition_name is not None:
                operands.append(partition_id_tensor())
            outs = _bass_exec_p.bind(
                *operands,
                out_avals=tuple(out_avals),
                in_names=tuple(all_in),
                out_names=tuple(out_names),
                lowering_input_output_aliases=(),
                sim_require_finite=True,
                sim_require_nnan=True,
                nc=nc,
            )
            return tuple(outs)

        devices = jax.devices()[:n_cores]
        mesh = Mesh(np.asarray(devices), ("core",))
        n_params = len(in_names)
        in_specs = (PartitionSpec("core"),) * (n_params + len(out_names))
        out_specs = (PartitionSpec("core"),) * len(out_names)
        self.fn = jax.jit(
            shard_map(_body, mesh=mesh, in_specs=in_specs,
                      out_specs=out_specs, check_rep=False),
            keep_unused=True,
        )
        self.sharding = NamedSharding(mesh, PartitionSpec("core"))
        self.zero_shapes = zero_shapes

    def run(self, in_maps):
        jax = self.jax
        per_core = [[np.asarray(m[n]) for n in self.in_names] for m in in_maps]
        dev_in = [
            jax.device_put(
                np.concatenate([per_core[c][i] for c in range(self.n_cores)], 0),
                self.sharding,
            )
            for i in range(len(self.in_names))
        ]
        dev_zero = [
            jax.device_put(
                np.zeros((self.n_cores * s[0], *s[1:]), d), self.sharding
            )
            for (s, d) in self.zero_shapes
        ]
        outs = self.fn(*dev_in, *dev_zero)
        jax.block_until_ready(outs)
        res = []
        for c in range(self.n_cores):
            d = {}
            for i, name in enumerate(self.out_names):
                full = np.asarray(outs[i])
                d[name] = full.reshape(self.n_cores, *self.out_avals[i].shape)[c]
            res.append(d)
        return res


_CACHE = {}


def kernel(x, w_qkv, b_qkv):
    x = np.asarray(x, np.float32)
    w_qkv = np.asarray(w_qkv, np.float32)
    b_qkv = np.asarray(b_qkv, np.float32)
    if "runner" not in _CACHE:
        _CACHE["runner"] = _Runner(_build())
    runner = _CACHE["runner"]
    in_maps = [
        _core_inputs(x[c // 4], w_qkv, b_qkv, c % 4) for c in range(8)
    ]
    res = runner.run(in_maps)
    out = np.empty((B, T, C), np.float32)
    for c in range(8):
        b, g = c // 4, c % 4
        out[b, :, 128 * g:128 * (g + 1)] = res[c]["out"]
    return out
